# revision 36
# baseline (speedup 1.0000x reference)
"""Multi-head attention on 8 Trainium2 NeuronCores.

Problem: B=2, L=2048, D=1024, N=16 heads, H=64.
Sharding: core i -> batch (i // 4), heads [4*(i%4), 4*(i%4)+4).
Each core: QKV projections for its 4 heads, full-seq attention,
partial output projection. Host sums the 4 partial projections per batch.

Device pipeline (per core), all layouts chosen so the contraction dim is
on partitions (no on-device transposes; host passes x pre-transposed):
  QT[e,l] = sum_d wq[d,e] xT[d,l]      (lhsT=wq chunk, rhs=xT chunk)
  KT[e,l] likewise; V[t,e] = sum_d xT[d,t-chunk] wv[d,e] (lhsT=xT, rhs=wv)
  LT[t,f] = sum_h KT[h,t] QT[h,f]      (per head)
  E = exp(LT)                          (ACT, PSUM->SBUF)
  [O_un.T ; norm] = [V_n | 1].T @ E:   lhsT=[V_n|ones][t,65], rhs=E[t,f]
  O.T = O_un.T * (1/norm) broadcast    (rank-1 matmul + 64-lane recip)
  out[l,d] += sum_h O.T[h,l-chunk] wo[h,d]

K=64 matmuls run at half rate on TRN2, so all contraction-64 matmuls
(logits, out-proj) are padded to K=128: the stationary operand keeps its
head's 64 rows and zeros elsewhere, making the moving operand's other
rows irrelevant. Matmuls run as float32r (full PE rate, ~TF32 rounding);
x and all weights are bf16 host-cast.
"""

import numpy as np

B, L, D = 2, 2048, 1024
NHEADS, HDIM = 16, 64
NCORES = 8
HPC = 4  # heads per core
E = HPC * HDIM  # 256
DCH = D // 128  # 8 d-chunks
TCH = L // 128  # 16 t/l chunks
FB = 1024  # f-block size in attention phase
NFB = L // FB
VW = HDIM + 1  # V' width per head (64 cols V + 1 ones col)

_CACHED_NC = None


def _build_nc():
    import concourse.mybir as mybir
    from concourse import bacc
    from concourse.tile import TileContext

    f32 = mybir.dt.float32
    f32r = mybir.dt.float32r
    bf16 = mybir.dt.bfloat16
    EXP = mybir.ActivationFunctionType.Exp

    nc = bacc.Bacc("TRN2", target_bir_lowering=False, num_devices=NCORES)

    xq = nc.declare_dram_parameter("xq", [D, L], bf16, isOutput=False)
    xk = nc.declare_dram_parameter("xk", [D, L], bf16, isOutput=False)
    xv = nc.declare_dram_parameter("xv", [D, L], bf16, isOutput=False)
    wq = nc.declare_dram_parameter("wq", [D, E], bf16, isOutput=False)
    wk = nc.declare_dram_parameter("wk", [D, E], bf16, isOutput=False)
    wv = nc.declare_dram_parameter("wv", [D, E], bf16, isOutput=False)
    wo = nc.declare_dram_parameter("wo", [E, D], bf16, isOutput=False)
    out = nc.declare_dram_parameter("out", [L, D], f32, isOutput=True)

    with TileContext(nc) as tc:
        with tc.tile_pool(name="persist", bufs=1) as cpool:
            # --- persistent SBUF tensors ---
            wq_sb = cpool.tile([128, DCH, E], bf16, tag="wq")
            wk_sb = cpool.tile([128, DCH, E], bf16, tag="wk")
            wv_sb = cpool.tile([128, DCH, E], bf16, tag="wv")
            # wo padded to 128 rows (64-127 zeroed) for K=128 out-proj
            wo_sb = cpool.tile([128, HPC, D], bf16, tag="wo")
            qt_sb = cpool.tile([128, 2, L], f32r, tag="qt")
            # kt zero-padded per head: head n's data in rows (n%2)*64..+64,
            # zeros in the other 64 rows -> K=128 logits at full rate
            kt_z = cpool.tile([128, HPC, L], f32r, tag="ktz")
            v_sb = cpool.tile([128, TCH, HPC * VW], bf16, tag="v")
            # ont padded: data rows 0-63, zeros 64-127 -> K=128 out-proj
            ont_z = cpool.tile([128, HPC, L], bf16, tag="ont")

            # ---------------- Phase 1: Q and V projections ----------------
            # x pool spans both phases (xk is consumed by the KT projection
            # that runs inside the attention scope).
            with tc.tile_pool(name="xp", bufs=2) as xpool:

                def load_x(x_dram):
                    xt = xpool.tile([128, DCH, L], bf16, tag="x")
                    xr = x_dram.rearrange("(c p) l -> p c l", p=128)
                    for d in range(DCH):
                        nc.sync.dma_start(out=xt[:, d, :], in_=xr[:, d, :])
                    return xt

                with (
                    tc.tile_pool(name="psA", bufs=4, space="PSUM") as psA,
                    tc.tile_pool(name="psV", bufs=4, space="PSUM") as psV,
                ):
                    nc.sync.dma_start(
                        out=wq_sb[:],
                        in_=wq.rearrange("(c p) e -> p c e", p=128),
                    )
                    xtq = load_x(xq)
                    # tiny constants early: zero column for kt_z stripes
                    # (DVE work during the Q projection)
                    cst = np.concatenate(
                        [
                            np.ones((128, 64), np.float32),
                            np.zeros((128, 1), np.float32),
                        ],
                        axis=1,
                    )
                    cst_dram = nc.inline_tensor(cst, name="cst")
                    cst_src = cst_dram.ap().bitcast(f32r)
                    zero_sb = cpool.tile([128, 1], f32r, tag="zero")
                    nc.sync.dma_start(out=zero_sb[:], in_=cst_src[:, 64:65])
                    for zn in range(HPC):
                        lo = 64 if zn % 2 == 0 else 0
                        nc.vector.tensor_copy(
                            out=kt_z[lo : lo + 64, zn, :],
                            in_=zero_sb[lo : lo + 64, 0:1].to_broadcast(
                                (64, L)
                            ),
                        )
                    nc.vector.memset(wo_sb[64:128, :, :], 0.0)
                    nc.vector.memset(ont_z[64:128, :, :], 0.0)

                    nc.sync.dma_start(
                        out=wv_sb[:],
                        in_=wv.rearrange("(c p) e -> p c e", p=128),
                    )

                    # QT: d-outer, (e,lb) grid in two passes of 4 banks
                    grid = [(e, lb) for e in range(2) for lb in range(L // 512)]
                    for half in range(2):
                        cells = grid[half * 4 : half * 4 + 4]
                        pss = [
                            psA.tile(
                                [128, 512], f32, tag="ps", name=f"q_{half}_{i}"
                            )
                            for i in range(len(cells))
                        ]
                        for d in range(DCH):
                            for ps, (e, lb) in zip(pss, cells):
                                nc.tensor.matmul(
                                    ps[:],
                                    wq_sb[:, d, e * 128 : (e + 1) * 128],
                                    xtq[:, d, lb * 512 : (lb + 1) * 512],
                                    start=(d == 0),
                                    stop=(d == DCH - 1),
                                )
                        for ps, (e, lb) in zip(pss, cells):
                            nc.vector.tensor_copy(
                                out=qt_sb[:, e, lb * 512 : (lb + 1) * 512],
                                in_=ps[:],
                            )

                    xtv = load_x(xv)
                    nc.vector.memset(
                        v_sb[:].rearrange("p t (n c) -> p t n c", n=HPC)[
                            :, :, :, HDIM : HDIM + 1
                        ],
                        1.0,
                    )
                    nc.sync.dma_start(
                        out=wo_sb[0:64, :, :],
                        in_=wo.rearrange("(n p) d -> p n d", p=64),
                    )

                    # V: d-outer in 4 passes of 4 t-chunks
                    for vpass in range(4):
                        ts4 = range(vpass * 4, vpass * 4 + 4)
                        vps = [
                            psV.tile(
                                [128, E], f32, tag="psv", name=f"v_{vpass}_{i}"
                            )
                            for i in range(4)
                        ]
                        for d in range(DCH):
                            for ps, t in zip(vps, ts4):
                                nc.tensor.matmul(
                                    ps[:],
                                    xtv[:, d, t * 128 : (t + 1) * 128],
                                    wv_sb[:, d, :],
                                    start=(d == 0),
                                    stop=(d == DCH - 1),
                                )
                        for ps, t in zip(vps, ts4):
                            # ACT is idle here; keeps DVE free for kt stores
                            nc.scalar.copy(
                                out=v_sb[:, t, :].rearrange(
                                    "p (n c) -> p n c", n=HPC
                                )[:, :, 0:HDIM],
                                in_=ps[:].rearrange("p (n c) -> p n c", n=HPC),
                            )

                    nc.sync.dma_start(
                        out=wk_sb[:],
                        in_=wk.rearrange("(c p) e -> p c e", p=128),
                    )
                    xtk = load_x(xk)  # reuses xq's slot

                # ---- Phase 2(+KT,+out-proj): ACT-paced global pump ----
                # The logits->exp stream runs through a global cursor with
                # ~PREF tiles of lookahead, so the ACT engine never stalls
                # on PE hiccups, head boundaries, or the KT projection
                # (which runs here, interleaved, on its own PSUM pool).
                PREF = 10
                from collections import deque

                with (
                    tc.tile_pool(name="psL", bufs=2, space="PSUM") as psL,
                    tc.tile_pool(name="ep", bufs=PREF) as epool,
                    tc.tile_pool(name="rp", bufs=2) as rpool,
                    tc.tile_pool(name="bp", bufs=2) as bpool,
                    tc.tile_pool(name="op", bufs=3) as opool,
                ):
                    heads = [(fb, n) for fb in range(NFB) for n in range(HPC)]
                    cursor = [0]
                    pslq = deque()
                    etq = deque()

                    def pump_logits():
                        k = cursor[0]
                        if k >= len(heads) * TCH:
                            return
                        cursor[0] += 1
                        fb, n = heads[k // TCH]
                        t = k % TCH
                        psl = psL.tile(
                            [128, FB], f32, tag="psl", name=f"psl_{k}"
                        )
                        ch, f0 = n // 2, fb * FB
                        for h2 in range(FB // 512):
                            nc.tensor.matmul(
                                psl[:, h2 * 512 : (h2 + 1) * 512],
                                kt_z[:, n, t * 128 : (t + 1) * 128],
                                qt_sb[
                                    :, ch, f0 + h2 * 512 : f0 + (h2 + 1) * 512
                                ],
                                start=True,
                                stop=True,
                            )
                        pslq.append((k, psl))

                    def pump_exp():
                        if not pslq:
                            return
                        k, psl = pslq.popleft()
                        et = epool.tile([128, FB], bf16, tag="e", name=f"et_{k}")
                        nc.scalar.activation(et[:], psl[:], EXP)
                        etq.append(et)

                    pending = []

                    def outproj_group(lc, db, on_act=False):
                        ps = psO.tile(
                            [128, 512], f32, tag="pso", name=f"ps3_{lc}_{db}"
                        )
                        for n in range(HPC):
                            nc.tensor.matmul(
                                ps[:],
                                ont_z[:, n, lc * 128 : (lc + 1) * 128],
                                wo_sb[:, n, db * 512 : (db + 1) * 512],
                                start=(n == 0),
                                stop=(n == HPC - 1),
                            )
                        ot = opool.tile(
                            [128, 512], f32, tag="o", name=f"ot_{lc}_{db}"
                        )
                        if on_act:  # drain: ACT is idle after the last exp
                            nc.scalar.copy(out=ot[:], in_=ps[:])
                        else:
                            nc.vector.tensor_copy(out=ot[:], in_=ps[:])
                        nc.sync.dma_start(
                            out=out[
                                lc * 128 : (lc + 1) * 128,
                                db * 512 : (db + 1) * 512,
                            ],
                            in_=ot[:],
                        )

                    # KT: half 0 (heads 0/1) plain; half 1 interleaved with
                    # the first PREF logits+exp of head 0 so the ACT spins
                    # up while KT finishes
                    grid = [(e, lb) for e in range(2) for lb in range(L // 512)]

                    globals_psK = [None]

                    def kt_half(half, interleave):
                        cells = grid[half * 4 : half * 4 + 4]
                        pss = [
                            globals_psK[0].tile(
                                [128, 512], f32, tag="psk", name=f"k_{half}_{i}"
                            )
                            for i in range(len(cells))
                        ]
                        for d in range(DCH):
                            for ps, (e, lb) in zip(pss, cells):
                                nc.tensor.matmul(
                                    ps[:],
                                    wk_sb[:, d, e * 128 : (e + 1) * 128],
                                    xtk[:, d, lb * 512 : (lb + 1) * 512],
                                    start=(d == 0),
                                    stop=(d == DCH - 1),
                                )
                            if interleave and d % 2 == 1:
                                pump_logits()
                                pump_exp()
                        for ps, (e, lb) in zip(pss, cells):
                            sl = slice(lb * 512, (lb + 1) * 512)
                            nc.vector.tensor_copy(
                                out=kt_z[0:64, 2 * e, sl], in_=ps[0:64, :]
                            )
                            nc.vector.tensor_copy(
                                out=kt_z[64:128, 2 * e + 1, sl],
                                in_=ps[64:128, :],
                            )

                    with tc.tile_pool(name="psK", bufs=4, space="PSUM") as psK:
                        globals_psK[0] = psK
                        kt_half(0, False)
                        kt_half(1, True)
                        while cursor[0] < PREF:
                            pump_logits()
                            pump_exp()

                    psO_cm = tc.tile_pool(name="psO", bufs=2, space="PSUM")
                    psO = psO_cm.__enter__()
                    for hi, (fb, n) in enumerate(heads):
                        f0 = fb * FB
                        pso = psO.tile(
                            [VW, FB], f32, tag="pso", name=f"pso_{fb}_{n}"
                        )
                        for t in range(TCH):
                            pump_logits()
                            pump_exp()
                            et = etq.popleft()
                            for h2 in range(FB // 512):
                                nc.tensor.matmul(
                                    pso[:, h2 * 512 : (h2 + 1) * 512],
                                    v_sb[:, t, n * VW : (n + 1) * VW],
                                    et[:, h2 * 512 : (h2 + 1) * 512],
                                    start=(t == 0),
                                    stop=(t == TCH - 1),
                                )
                            if t % 4 == 2 and pending:
                                outproj_group(*pending.pop(0))

                        # normalize: O.T = O_un.T * (1/norm); norm row ->
                        # [32,32] via DMA (recip cost tracks free size),
                        # recip, back to a row, gpsimd broadcast, multiply
                        rt = rpool.tile([VW, FB], f32, tag="r")
                        nc.vector.tensor_copy(
                            out=rt[HDIM : HDIM + 1, :],
                            in_=pso[HDIM : HDIM + 1, :],
                        )
                        rsq = rpool.tile([32, 2 * (FB // 32)], f32, tag="rsq")
                        nc.sync.dma_start(
                            out=rsq[:, 0 : FB // 32],
                            in_=rt[HDIM : HDIM + 1, :],
                        )
                        with nc.allow_low_precision(reason="softmax recip"):
                            nc.vector.reciprocal(
                                out=rsq[:, FB // 32 :],
                                in_=rsq[:, 0 : FB // 32],
                            )
                        rt2 = rpool.tile([1, FB], f32, tag="r2")
                        nc.sync.dma_start(out=rt2[:], in_=rsq[:, FB // 32 :])
                        bt = bpool.tile([64, FB], f32, tag="b")
                        nc.gpsimd.partition_broadcast(
                            bt[:], rt2[:], channels=64
                        )
                        nc.vector.tensor_mul(
                            out=ont_z[0:HDIM, n, f0 : f0 + FB],
                            in0=pso[0:HDIM, :],
                            in1=bt[:],
                        )

                        if n == HPC - 1:
                            pending += [
                                (lc, db)
                                for lc in range(
                                    fb * (FB // 128), (fb + 1) * (FB // 128)
                                )
                                for db in range(D // 512)
                            ]

                    # drain the last f-block's out-projection
                    for lc, db in pending:
                        outproj_group(lc, db, on_act=True)
                    psO_cm.__exit__(None, None, None)

    nc.compile()
    return nc


def _get_nc():
    global _CACHED_NC
    if _CACHED_NC is None:
        _CACHED_NC = _build_nc()
    return _CACHED_NC


def _make_in_maps(query_input, key_input, value_input, Wq, Wk, Wv, Wo):
    import ml_dtypes

    bf16 = ml_dtypes.bfloat16
    scale = np.float32(HDIM) ** np.float32(-0.5)

    xT = {}
    for b in range(B):
        xT[("q", b)] = np.ascontiguousarray(query_input[b].T).astype(bf16)
        xT[("k", b)] = np.ascontiguousarray(key_input[b].T).astype(bf16)
        xT[("v", b)] = np.ascontiguousarray(value_input[b].T).astype(bf16)

    in_maps = []
    for core in range(NCORES):
        b = core // 4
        g = core % 4
        hs = slice(g * HPC, (g + 1) * HPC)
        in_maps.append(
            {
                "xq": xT[("q", b)],
                "xk": xT[("k", b)],
                "xv": xT[("v", b)],
                "wq": np.ascontiguousarray(
                    (Wq[:, hs, :] * scale).reshape(D, E)
                ).astype(bf16),
                "wk": np.ascontiguousarray(Wk[:, hs, :].reshape(D, E)).astype(bf16),
                "wv": np.ascontiguousarray(Wv[:, hs, :].reshape(D, E)).astype(bf16),
                "wo": np.ascontiguousarray(Wo[hs].reshape(E, D)).astype(bf16),
            }
        )
    return in_maps


def _combine(results):
    out = np.empty((B, L, D), dtype=np.float32)
    for b in range(B):
        acc = results[b * 4]["out"].astype(np.float32)
        for g in range(1, 4):
            acc = acc + results[b * 4 + g]["out"]
        out[b] = acc
    return out


def kernel(query_input, key_input, value_input, Wq, Wk, Wv, Wo):
    from concourse.bass_utils import run_bass_kernel_spmd

    nc = _get_nc()
    in_maps = _make_in_maps(query_input, key_input, value_input, Wq, Wk, Wv, Wo)
    res = run_bass_kernel_spmd(nc, in_maps, core_ids=list(range(NCORES)))
    return _combine(res.results)


if __name__ == "__main__":
    rng = np.random.default_rng(0)
    inputs = {
        "query_input": rng.standard_normal((B, L, D), dtype=np.float32),
        "key_input": rng.standard_normal((B, L, D), dtype=np.float32),
        "value_input": rng.standard_normal((B, L, D), dtype=np.float32),
        "Wq": rng.standard_normal((D, NHEADS, HDIM), dtype=np.float32) * 0.03,
        "Wk": rng.standard_normal((D, NHEADS, HDIM), dtype=np.float32) * 0.03,
        "Wv": rng.standard_normal((D, NHEADS, HDIM), dtype=np.float32) * 0.03,
        "Wo": rng.standard_normal((NHEADS, HDIM, D), dtype=np.float32) * 0.03,
    }
    out = kernel(**inputs)
    print("kernel output", out.shape, out.dtype, float(np.abs(out).mean()))
